# revision 1
# baseline (speedup 1.0000x reference)
"""Trainium2 distributed kernel for the AppearanceReconstruction loss.

Math note (exact identity, not an approximation): the MAE shuffle/gather in
the reference collapses — restored[b,p] is appearance_tokens[b,p] on kept
slots (which the mask multiplies by zero) and mask_token on masked slots.
Every row has exactly num_masked = 2 masked slots, and the decoder output at
a masked slot is the same single vector r = MLP(LN(mask_token)) for all
(b,p). Hence

    loss = 2 * sum_b mean_c((r_c - pooled[b,c])^2) / (256 + 1e-8)
    pooled[b] = mean_n target_features[b,n,:]

The memory-bound part (reading all of target_features, 402 MB) runs on the 8
NeuronCores, data-parallel over B (16 rows per core). Each core streams its
50 MB shard through SBUF in 3 MB tiles and reduces over N with TensorEngine
matmuls against one-hot columns (scaled by 1/N), accumulating the 16 row
means into a [16, 768] PSUM tile. A short vector-engine epilogue computes
sum_c (pooled - r)^2 per row; the host sums the 8x16 partials.
"""

import math

import numpy as np

B, N, C = 128, 1024, 768
NCORES = 8
BPC = B // NCORES  # rows per core
PPB = 128  # SBUF partitions per row-tile
NSUB = N // PPB  # n-rows folded into each partition's free dim
FREE = NSUB * C  # 6144 floats per partition per row-tile
LN_EPS = 1e-5

_CACHE = {}

# kernel structure knobs (A/B-tested on hardware; defaults = best measured)
_VARIANT = {
    "last_split": True,
    "out_ring": "scalar",
    "ring_alt": False,
    "head_split": False,
    "bufs": 6,
}


def _build():
    import concourse.bass as bass  # noqa: F401
    import concourse.tile as tile
    from concourse import bacc, mybir

    f32 = mybir.dt.float32
    f32r = mybir.dt.float32r
    AL = mybir.AluOpType
    AX = mybir.AxisListType

    nc = bacc.Bacc(
        "TRN2", target_bir_lowering=False, debug=False, num_devices=NCORES
    )
    tf = nc.dram_tensor("tf", [BPC, PPB, FREE], f32r, kind="ExternalInput")
    negr = nc.dram_tensor("negr", [1, C], f32r, kind="ExternalInput")
    ones16 = nc.dram_tensor("ones16", [1, BPC], f32r, kind="ExternalInput")
    emat = nc.dram_tensor("emat", [PPB, BPC * BPC], f32r, kind="ExternalInput")
    out = nc.dram_tensor("out", [BPC, 1], f32, kind="ExternalOutput")

    with tile.TileContext(nc) as tc:
        with (
            tc.tile_pool(name="consts", bufs=1) as cpool,
            tc.tile_pool(name="data", bufs=_VARIANT["bufs"]) as dpool,
            tc.tile_pool(name="epi", bufs=1) as epool,
            tc.tile_pool(name="psum", bufs=1, space="PSUM") as ppool,
        ):
            # issue the first data DMAs before the tiny const loads so the
            # big stream starts as early as possible; with head_split the
            # first row goes as two halves issued from BOTH HWDGE rings so
            # their descriptor generation runs in parallel at t=0
            hhalf = NSUB // 2
            if _VARIANT["head_split"]:
                t0_tile = dpool.tile([PPB, hhalf * C], f32r, tag="data")
                nc.sync.dma_start(out=t0_tile[:], in_=tf.ap()[0, :, 0 : hhalf * C])
                t0b_tile = dpool.tile([PPB, hhalf * C], f32r, tag="data")
                nc.scalar.dma_start(
                    out=t0b_tile[:], in_=tf.ap()[0, :, hhalf * C : FREE]
                )
            else:
                t0_tile = dpool.tile([PPB, FREE], f32r, tag="data")
                nc.sync.dma_start(out=t0_tile[:], in_=tf.ap()[0])
                t0b_tile = None

            # const loads go on the ACT HWDGE ring so the SP ring's first job
            # is the 3 MB stream itself
            emat_sb = cpool.tile([PPB, BPC * BPC], f32r)
            nc.scalar.dma_start(out=emat_sb[:], in_=emat.ap())
            negr_sb = cpool.tile([1, C], f32r)
            nc.scalar.dma_start(out=negr_sb[:], in_=negr.ap())
            ones16_sb = cpool.tile([1, BPC], f32r)
            nc.scalar.dma_start(out=ones16_sb[:], in_=ones16.ap())

            # single [16, 768] accumulator spanning two PSUM banks; each
            # matmul's out AP stays within one bank (512 | 256)
            ps = ppool.tile([BPC, C], f32)

            # (row, sub_lo, sub_hi) chunks; full 3 MB rows keep the DMA
            # stream at peak rate, only the last row is halved so the
            # post-final-DMA PE tail is half a row
            half = NSUB // 2
            quart = NSUB // 4
            if _VARIANT["head_split"]:
                chunks = [(0, 0, half), (0, half, NSUB)]
            else:
                chunks = [(0, 0, NSUB)]
            chunks += [(b, 0, NSUB) for b in range(1, BPC - 1)]
            if _VARIANT["last_split"]:
                chunks += [
                    (BPC - 1, 0, half),
                    (BPC - 1, half, half + quart),
                    (BPC - 1, half + quart, NSUB),
                ]
            else:
                chunks += [(BPC - 1, 0, NSUB)]

            for ci, (b, lo, hi) in enumerate(chunks):
                if ci == 0:
                    t = t0_tile
                elif ci == 1 and t0b_tile is not None:
                    t = t0b_tile
                else:
                    t = dpool.tile([PPB, (hi - lo) * C], f32r, tag="data")
                    # optionally alternate the two HWDGE rings (SP/ACT) so
                    # descriptor generation of consecutive transfers overlaps;
                    # the final two chunks stay on one ring to preserve their
                    # completion order (the PE tail depends on the last chunk
                    # alone finishing last)
                    if _VARIANT["ring_alt"] and ci < len(chunks) - 2:
                        eng = nc.sync if ci % 2 == 0 else nc.scalar
                    else:
                        eng = nc.sync
                    eng.dma_start(
                        out=t[:], in_=tf.ap()[b, :, lo * C : hi * C]
                    )
                # float32r: same 4-byte layout, 4x faster PE streaming; the
                # reduced-precision multiply is far inside the loss tolerance.
                lhsT = emat_sb[:, b * BPC : (b + 1) * BPC]
                first = ci == 0
                last = ci == len(chunks) - 1
                for sub in range(lo, hi):
                    nc.tensor.matmul(
                        ps[:, 0:512],
                        lhsT,
                        t[:, (sub - lo) * C : (sub - lo) * C + 512],
                        start=first and sub == lo,
                        stop=last and sub == hi - 1,
                    )
                for sub in range(lo, hi):
                    nc.tensor.matmul(
                        ps[:, 512:768],
                        lhsT,
                        t[:, (sub - lo) * C + 512 : (sub - lo + 1) * C],
                        start=first and sub == lo,
                        stop=last and sub == hi - 1,
                    )
                if ci == 0:
                    # fold the "- r" into the accumulation: one K=1 matmul
                    # adds -r_c to every row, early so it is off the tail.
                    # PSUM then holds (pooled_mean - r) directly and the
                    # epilogue shrinks to square + reduce.
                    nc.tensor.matmul(
                        ps[:, 0:512],
                        ones16_sb[:],
                        negr_sb[:, 0:512],
                        start=False,
                        stop=False,
                    )
                    nc.tensor.matmul(
                        ps[:, 512:768],
                        ones16_sb[:],
                        negr_sb[:, 512:768],
                        start=False,
                        stop=False,
                    )

            # one ACT instruction: square every element of (pooled - r) and
            # row-sum into s — single PSUM read, runs on the idle ACT engine
            sq = epool.tile([BPC, C], f32)
            s = epool.tile([BPC, 1], f32)
            nc.scalar.activation(
                out=sq[:],
                in_=ps[:],
                func=mybir.ActivationFunctionType.Square,
                accum_out=s[:],
            )
            # output DMA on the ACT HWDGE ring so it never queues behind the
            # SP ring's bulk data stream
            out_eng = nc.scalar if _VARIANT["out_ring"] == "scalar" else nc.sync
            out_eng.dma_start(out=out.ap(), in_=s[:])

    nc.compile()
    return nc


def _get_nc():
    nc = _CACHE.get("nc")
    if nc is None:
        nc = _build()
        _CACHE["nc"] = nc
    return nc


def _host_r(mask_token, ln_w, ln_b, W1, b1, W2, b2):
    """r = Linear2(gelu_exact(Linear1(LayerNorm(mask_token)))) — one 768-vec."""
    mt = np.asarray(mask_token, np.float64).reshape(C)
    mu = mt.mean()
    var = ((mt - mu) ** 2).mean()
    x = (mt - mu) / np.sqrt(var + LN_EPS) * np.asarray(ln_w, np.float64) + np.asarray(
        ln_b, np.float64
    )
    h = x @ np.asarray(W1, np.float64) + np.asarray(b1, np.float64)
    erf = np.frompyfunc(math.erf, 1, 1)
    g = h * 0.5 * (1.0 + erf(h / math.sqrt(2.0)).astype(np.float64))
    r = g @ np.asarray(W2, np.float64) + np.asarray(b2, np.float64)
    return r.astype(np.float32)


def kernel(
    appearance_tokens,
    target_features,
    noise,
    mask_token,
    ln_w,
    ln_b,
    W1,
    b1,
    W2,
    b2,
):
    from concourse.bass_utils import run_bass_kernel_spmd

    nc = _get_nc()

    r = _host_r(mask_token, ln_w, ln_b, W1, b1, W2, b2)
    in_maps = [
        {"tf": tfull_i, **_const_inputs(r)} for tfull_i in _shard_tf(target_features)
    ]

    res = run_bass_kernel_spmd(nc, in_maps, list(range(NCORES)))
    total = 0.0
    for i in range(NCORES):
        total += float(np.asarray(res.results[i]["out"], np.float64).sum())

    loss = 2.0 * total / C / (256.0 + 1e-8)
    return np.float32(loss)


def _const_inputs(r):
    """Constant device inputs derived from the decoder vector r."""
    negr = np.ascontiguousarray(-r.reshape(1, C), np.float32)
    ones16 = np.ones((1, BPC), np.float32)
    # emat[:, b*16+m] = 1/N if m == b else 0 — one-hot columns scaled so the
    # partition-reduction matmul lands mean_n directly in PSUM row b.
    emat = np.zeros((PPB, BPC * BPC), np.float32)
    for b in range(BPC):
        emat[:, b * BPC + b] = 1.0 / N
    return {"negr": negr, "ones16": ones16, "emat": emat}


def _shard_tf(target_features):
    return np.ascontiguousarray(target_features, np.float32).reshape(
        NCORES, BPC, PPB, FREE
    )



# revision 3
# speedup vs baseline: 2.8012x; 2.8012x over previous
"""Trainium2 distributed kernel for the AppearanceReconstruction loss.

Math note (exact identity, not an approximation): the MAE shuffle/gather in
the reference collapses — restored[b,p] is appearance_tokens[b,p] on kept
slots (which the mask multiplies by zero) and mask_token on masked slots.
Every row has exactly num_masked = 2 masked slots, and the decoder output at
a masked slot is the same single vector r = MLP(LN(mask_token)) for all
(b,p). Hence

    loss = 2 * sum_b mean_c((r_c - pooled[b,c])^2) / (256 + 1e-8)
    pooled[b] = mean_n target_features[b,n,:]

The memory-bound part (reading all of target_features) runs on the 8
NeuronCores, data-parallel over B (16 rows per core). target_features is
cast to fp8 e4m3 on the host before upload, quartering the HBM stream
(12.6 MB/core); the e4m3 quantization noise averages out over the
1024-token mean to ~3e-5 relative on the loss. Each core's shard is
DMA'd into SBUF in 3.1 MB chunks (it fits entirely, so no buffer reuse)
and reduced over tokens with DoubleRow fp8 TensorEngine matmuls — 2
tokens per PE column-cycle — against one-hot row-selector weights,
accumulating token sums into a [16, 768] PSUM tile. A K=1 f32r matmul
folds -N*r into the same accumulation, so PSUM ends as N*(pooled - r)
and the epilogue is one ACT Square(scale=1/N)+row-sum. The host sums the
8x16 partials.
"""

import math

import numpy as np

B, N, C = 128, 1024, 768
NCORES = 8
BPC = B // NCORES  # rows per core
PPB = 128  # SBUF partitions
LN_EPS = 1e-5

NCH = 4  # big chunks per core (4 rows each)
RPC = BPC // NCH  # rows per chunk
CHB = RPC * N * C // PPB  # bytes per partition per chunk (24576)
TPP = CHB // C  # tokens per partition per chunk (32)
# last chunk is split along the free dim so the PE tail after the final
# DMA is a single q-slice (1536 B/partition)
LAST_SPLITS = [(0, CHB // 2), (CHB // 2, 3 * CHB // 4),
               (3 * CHB // 4, CHB - 2 * C), (CHB - 2 * C, CHB)]

_CACHE = {}


def _build():
    import concourse.bass as bass  # noqa: F401
    import concourse.tile as tile
    from concourse import bacc, mybir

    f32 = mybir.dt.float32
    f32r = mybir.dt.float32r
    f8 = mybir.dt.float8e4

    nc = bacc.Bacc(
        "TRN2", target_bir_lowering=False, debug=False, num_devices=NCORES
    )
    tf = nc.dram_tensor("tf", [NCH, PPB, CHB], f8, kind="ExternalInput")
    negnr = nc.dram_tensor("negnr", [1, C], f32r, kind="ExternalInput")
    ones16 = nc.dram_tensor("ones16", [1, BPC], f32r, kind="ExternalInput")
    emat = nc.dram_tensor("emat", [PPB, NCH * 2 * BPC], f8, kind="ExternalInput")
    out = nc.dram_tensor("out", [BPC, 1], f32, kind="ExternalOutput")

    DR = mybir.MatmulPerfMode.DoubleRow

    with tile.TileContext(nc) as tc:
        with (
            tc.tile_pool(name="consts", bufs=1) as cpool,
            tc.tile_pool(name="data", bufs=NCH - 1 + len(LAST_SPLITS)) as dpool,
            tc.tile_pool(name="epi", bufs=1) as epool,
            tc.tile_pool(name="psum", bufs=1, space="PSUM") as ppool,
        ):
            # (chunk, lo, hi) pieces; full 3.1 MB chunks keep the DMA stream
            # at peak rate, only the last chunk is split so the post-final-DMA
            # PE tail is one q-slice
            chunks = [(cb, 0, CHB) for cb in range(NCH - 1)]
            chunks += [(NCH - 1, lo, hi) for lo, hi in LAST_SPLITS]

            # issue the first data DMA before the tiny const loads so the
            # big stream starts as early as possible
            tiles = []
            t0 = dpool.tile([PPB, CHB], f8, tag="data")
            nc.sync.dma_start(out=t0[:], in_=tf.ap()[0])
            tiles.append(t0)

            # const loads go on the ACT HWDGE ring so the SP ring only
            # carries the bulk stream
            emat_sb = cpool.tile([PPB, NCH * 2 * BPC], f8)
            nc.scalar.dma_start(out=emat_sb[:], in_=emat.ap())
            negnr_sb = cpool.tile([1, C], f32r)
            nc.scalar.dma_start(out=negnr_sb[:], in_=negnr.ap())
            ones16_sb = cpool.tile([1, BPC], f32r)
            nc.scalar.dma_start(out=ones16_sb[:], in_=ones16.ap())

            for cb, lo, hi in chunks[1:]:
                t = dpool.tile([PPB, hi - lo], f8, tag="data")
                nc.sync.dma_start(out=t[:], in_=tf.ap()[cb, :, lo:hi])
                tiles.append(t)

            # single [16, 768] f32 accumulator spanning two PSUM banks; each
            # matmul's out AP stays within one bank (512 | 256)
            ps = ppool.tile([BPC, C], f32)

            for ci, ((cb, lo, hi), t) in enumerate(zip(chunks, tiles)):
                # one-hot row-selector weights for this chunk: [128, 2, 16],
                # w[p, j, m] = 1 iff m == cb*RPC + p//32
                lhsT = emat_sb[:, cb * 2 * BPC : (cb + 1) * 2 * BPC].rearrange(
                    "p (j m) -> p j m", j=2
                )
                first = ci == 0
                last = ci == len(chunks) - 1
                nq = (hi - lo) // (2 * C)  # DoubleRow q-slices in this piece
                for q in range(nq):
                    # rhs [128, 2, c]: j picks the 2nd token of the pair
                    pair = t[:, q * 2 * C : (q + 1) * 2 * C].rearrange(
                        "p (j c) -> p j c", j=2
                    )
                    nc.tensor.matmul(
                        ps[:, 0:512],
                        lhsT[:, :, :],
                        pair[:, :, 0:512],
                        start=first and q == 0,
                        stop=last and q == nq - 1,
                        perf_mode=DR,
                    )
                    nc.tensor.matmul(
                        ps[:, 512:768],
                        lhsT[:, :, :],
                        pair[:, :, 512:768],
                        start=first and q == 0,
                        stop=last and q == nq - 1,
                        perf_mode=DR,
                    )
                if ci == 0:
                    # fold the "- N*r" into the accumulation: one K=1 f32r
                    # matmul adds -N*r_c to every row, early so it is off the
                    # tail. PSUM then holds N*(pooled_mean - r) directly.
                    nc.tensor.matmul(
                        ps[:, 0:512],
                        ones16_sb[:],
                        negnr_sb[:, 0:512],
                        start=False,
                        stop=False,
                        skip_group_check=True,
                    )
                    nc.tensor.matmul(
                        ps[:, 512:768],
                        ones16_sb[:],
                        negnr_sb[:, 512:768],
                        start=False,
                        stop=False,
                        skip_group_check=True,
                    )

            # one ACT instruction: square every element of (pooled - r) and
            # row-sum into s — single PSUM read, runs on the idle ACT engine
            sq = epool.tile([BPC, C], f32)
            s = epool.tile([BPC, 1], f32)
            nc.scalar.activation(
                out=sq[:],
                in_=ps[:],
                func=mybir.ActivationFunctionType.Square,
                scale=1.0 / N,
                accum_out=s[:],
            )
            # output DMA on the ACT HWDGE ring so it never queues behind the
            # SP ring's bulk data stream
            nc.scalar.dma_start(out=out.ap(), in_=s[:])

    nc.compile()
    return nc


def _get_nc():
    nc = _CACHE.get("nc")
    if nc is None:
        nc = _build()
        _CACHE["nc"] = nc
    return nc


def _host_r(mask_token, ln_w, ln_b, W1, b1, W2, b2):
    """r = Linear2(gelu_exact(Linear1(LayerNorm(mask_token)))) — one 768-vec."""
    mt = np.asarray(mask_token, np.float64).reshape(C)
    mu = mt.mean()
    var = ((mt - mu) ** 2).mean()
    x = (mt - mu) / np.sqrt(var + LN_EPS) * np.asarray(ln_w, np.float64) + np.asarray(
        ln_b, np.float64
    )
    h = x @ np.asarray(W1, np.float64) + np.asarray(b1, np.float64)
    erf = np.frompyfunc(math.erf, 1, 1)
    g = h * 0.5 * (1.0 + erf(h / math.sqrt(2.0)).astype(np.float64))
    r = g @ np.asarray(W2, np.float64) + np.asarray(b2, np.float64)
    return r.astype(np.float32)


def kernel(
    appearance_tokens,
    target_features,
    noise,
    mask_token,
    ln_w,
    ln_b,
    W1,
    b1,
    W2,
    b2,
):
    from concourse.bass_utils import run_bass_kernel_spmd

    nc = _get_nc()

    r = _host_r(mask_token, ln_w, ln_b, W1, b1, W2, b2)
    in_maps = [
        {"tf": tfull_i, **_const_inputs(r)} for tfull_i in _shard_tf(target_features)
    ]

    res = run_bass_kernel_spmd(nc, in_maps, list(range(NCORES)))
    total = 0.0
    for i in range(NCORES):
        total += float(np.asarray(res.results[i]["out"], np.float64).sum())

    loss = 2.0 * total / C / (256.0 + 1e-8)
    return np.float32(loss)


def _const_inputs(r):
    """Constant device inputs derived from the decoder vector r."""
    import ml_dtypes

    negnr = np.ascontiguousarray(-float(N) * r.reshape(1, C), np.float32)
    ones16 = np.ones((1, BPC), np.float32)
    # emat[p, cb*32 + j*16 + m] = 1 iff m == cb*RPC + p//(PPB//RPC): one-hot
    # DoubleRow row-selector weights (both j halves identical)
    emat = np.zeros((PPB, NCH * 2 * BPC), np.float32)
    p = np.arange(PPB)
    for cb in range(NCH):
        m = cb * RPC + p // (PPB // RPC)
        for j in range(2):
            emat[p, cb * 2 * BPC + j * BPC + m] = 1.0
    return {
        "negnr": negnr,
        "ones16": ones16,
        "emat": emat.astype(ml_dtypes.float8_e4m3),
    }


def _shard_tf(target_features):
    import ml_dtypes

    x8 = np.ascontiguousarray(target_features, np.float32).astype(
        ml_dtypes.float8_e4m3
    )
    return x8.reshape(NCORES, NCH, PPB, CHB)
